# revision 10
# baseline (speedup 1.0000x reference)
"""Trainium2 Bass kernel for ActivationPNASimplifiedLayer (GNN message passing).

Strategy (8 NeuronCores, SPMD, full inputs in / full output out):
  * Host (index-only): deal nodes to (tile, partition, core) sorted by
    (-pairs, singles); per round/class build L1 gather entry lists with a
    node's same-class edges adjacent (pair halves consecutive); L2 index
    tables fetch 512B *pairs* (2 staged rows per descriptor) plus singles.
  * Device: L1 dma_gather reads raw h rows straight from HBM per class
    quarter (int16-local indices); an in-place DVE pass applies norm[src]
    (host-reindexed values); a j-major restage writes staging DRAM so
    pair partners are adjacent; L2 dma_gather pulls pairs/singles into
    slot-major [p, s*T + t] layouts; segment sum/sumsq/max/min are
    strided tensor_reduce over the s axis with dup-pad corrections from
    first-pair/first-single slices; squares are computed in place after
    the first reduces (no extra SBUF).
  * Own-node term uses host-permuted h rows (contiguous DMA), PNA scalers
    and 13-way mean are batched wide ops; BatchNorm uses a ones-matmul
    partition reduction + 512B AllReduce as before.
"""

import math
import os

import numpy as np

# ---------------------------------------------------------------- constants
N_NODES = 100000
N_EDGES = 1200000
FEAT = 64
P = 128
NCORES = 8
NQ = 4
QROWS = N_NODES // NQ
AVG_D_LOG = float(np.log(13.0))
EPS_STD = 1e-5
EPS_BN = 1e-5
L16 = 16
TOK = 10 ** 7
NQUEUES = 4

_CACHE = {}
LAST_RESULTS = None


def _wrap16(vals):
    v = np.asarray(vals, np.int64)
    pad = (-len(v)) % L16
    if pad:
        v = np.concatenate([v, np.full(pad, -1, np.int64)])
    blk = v.reshape(-1, L16).T.astype(np.int16)
    return np.tile(blk, (P // L16, 1))


# ---------------------------------------------------------------- host side
def _build_schedule(src, dst, srmax=12000, tmax=10):
    src = np.asarray(src, np.int64)
    dst = np.asarray(dst, np.int64)
    deg = np.bincount(dst, minlength=N_NODES).astype(np.int64)
    row_start = np.zeros(N_NODES + 1, np.int64)
    np.cumsum(deg, out=row_start[1:])
    q_of = src // QROWS
    eorder = np.lexsort((q_of, dst))
    src_s = src[eorder]
    cc = np.bincount(dst * NQ + q_of, minlength=N_NODES * NQ).reshape(
        N_NODES, NQ)
    ccs = np.zeros((N_NODES, NQ + 1), np.int64)
    np.cumsum(cc, axis=1, out=ccs[:, 1:])
    pi0 = (cc // 2).sum(axis=1)
    lam0 = (cc % 2).sum(axis=1)
    seed_cand = (pi0 == 0) & (deg >= 1)
    pi_s = np.where(seed_cand, 1, pi0)
    lam_s = np.where(seed_cand, lam0 - 1, lam0)
    order = np.lexsort((lam_s, -pi_s))

    nt = math.ceil(N_NODES / (P * NCORES))
    rank = (np.arange(nt)[None, :] * (P * NCORES)
            + np.arange(P)[:, None] * NCORES)
    cores_ids = []
    for c in range(NCORES):
        r = rank + c
        vmc = r < N_NODES
        cores_ids.append((np.where(vmc, order[np.minimum(r, N_NODES - 1)], 0),
                          vmc))

    P2_t = np.zeros(nt, np.int64)
    S1_t = np.zeros(nt, np.int64)
    for t in range(nt):
        lo, hi = t * P * NCORES, min((t + 1) * P * NCORES, N_NODES)
        nd = order[lo:hi]
        P2_t[t] = pi_s[nd].max()
        S1_t[t] = (lam_s if P2_t[t] > 0 else lam0)[nd].max()

    rows_ct = np.zeros((NCORES, nt), np.int64)
    for c in range(NCORES):
        ids, vmc = cores_ids[c]
        d = np.where(vmc, deg[ids], 0)
        sd = np.where(vmc & (P2_t[None, :] > 0), seed_cand[ids], False)
        rows_ct[c] = (d + sd).sum(axis=0)

    # DP round boundaries under staging/SBUF caps
    rows_max = rows_ct.max(axis=0)
    FIXED = 2000
    PT_CAP = 48
    ST_CAP = 40
    INF = float("inf")
    f = [INF] * (nt + 1)
    arg = [0] * (nt + 1)
    f[0] = 0.0
    for hi in range(1, nt + 1):
        rows = 0
        p2 = s1 = 0
        for lo in range(hi - 1, -1, -1):
            rows += rows_max[lo]
            p2 = max(p2, P2_t[lo])
            s1 = max(s1, S1_t[lo])
            T = hi - lo
            if rows > srmax or T > tmax or p2 * T > PT_CAP or s1 * T > ST_CAP:
                break
            cost = f[lo] + FIXED + (p2 + s1) * T * P
            if cost < f[hi]:
                f[hi] = cost
                arg[hi] = lo
    rounds = []
    hi = nt
    while hi > 0:
        rounds.append((arg[hi], hi))
        hi = arg[hi]
    rounds.reverse()
    nrounds = len(rounds)
    rP2 = [int(P2_t[lo:hi].max()) for lo, hi in rounds]
    rS1 = [int(S1_t[lo:hi].max()) for lo, hi in rounds]

    # ---- pass 1: entry lists + tokens ----
    core_rd = [[None] * nrounds for _ in range(NCORES)]
    ncols_rq = np.zeros((nrounds, NQ), np.int64)
    for c in range(NCORES):
        ids, vmc = cores_ids[c]
        for ri, (rlo, rhi) in enumerate(rounds):
            T = rhi - rlo
            ent = [[] for _ in range(NQ)]
            ptok = np.full((max(rP2[ri], 1), T, P), -1, np.int64)
            stok = np.full((max(rS1[ri], 1), T, P), -1, np.int64)
            npair = np.zeros((T, P), np.int64)
            nsing = np.zeros((T, P), np.int64)
            seeded = np.zeros((T, P), np.int64)
            has_pairs = rP2[ri] > 0
            for tt in range(T):
                t = rlo + tt
                for p in range(P):
                    if not vmc[p, t] or deg[ids[p, t]] == 0:
                        continue
                    nd = ids[p, t]
                    base = row_start[nd]
                    do_seed = has_pairs and seed_cand[nd]
                    kp = ks = 0
                    for q in range(NQ):
                        nq_ = cc[nd, q]
                        if nq_ == 0:
                            continue
                        loc = (src_s[base + ccs[nd, q]:
                                     base + ccs[nd, q] + nq_] - q * QROWS)
                        if do_seed and kp == 0:
                            j = len(ent[q])
                            ent[q].extend([loc[0], loc[0]])
                            ptok[kp, tt, p] = q * TOK + j
                            kp += 1
                            seeded[tt, p] = 1
                            loc = loc[1:]
                        for i in range(len(loc) // 2):
                            j = len(ent[q])
                            ent[q].extend([loc[2 * i], loc[2 * i + 1]])
                            ptok[kp, tt, p] = q * TOK + j
                            kp += 1
                        if len(loc) % 2:
                            j = len(ent[q])
                            ent[q].append(loc[-1])
                            stok[ks, tt, p] = q * TOK + j
                            ks += 1
                    npair[tt, p] = kp
                    nsing[tt, p] = ks
            core_rd[c][ri] = dict(ent=ent, ptok=ptok, stok=stok,
                                  npair=npair, nsing=nsing, seeded=seeded)
            for q in range(NQ):
                ncols_rq[ri, q] = max(ncols_rq[ri, q],
                                      math.ceil(max(len(ent[q]), 1) / P))

    # ---- pass 2: index arrays + metas ----
    per_core = []
    for c in range(NCORES):
        ids, vmc = cores_ids[c]
        cidx_parts, nxid_parts, pidx_parts, sidx_parts = [], [], [], []
        l1cnt = np.zeros((nrounds, NQ + 2), np.int32)
        pcp = np.zeros((P, nt), np.float32)
        pcm1 = np.zeros((P, nt), np.float32)
        pcs1 = np.zeros((P, nt), np.float32)
        for ri, (rlo, rhi) in enumerate(rounds):
            T = rhi - rlo
            rd = core_rd[c][ri]
            cb = np.zeros(NQ + 1, np.int64)
            np.cumsum(ncols_rq[ri] * P, out=cb[1:])

            def srow(tok, cb=cb):
                return cb[tok // TOK] + tok % TOK

            srcof = np.zeros(int(cb[-1]), np.int64)
            for q in range(NQ):
                want = int(ncols_rq[ri, q] * P)
                if not rd["ent"][q]:
                    rd["ent"][q].append(0)
                l1cnt[ri, q] = len(rd["ent"][q])
                l1cnt[ri, NQ] = rP2[ri] * T * P
                l1cnt[ri, NQ + 1] = rS1[ri] * T * P
                il = np.full(want, -1, np.int64)
                il[:len(rd["ent"][q])] = rd["ent"][q]
                cidx_parts.append(_wrap16(il))
                srcof[cb[q]:cb[q] + len(rd["ent"][q])] = (
                    np.asarray(rd["ent"][q], np.int64) + q * QROWS)
            if rP2[ri] > 0:
                pl = np.zeros((rP2[ri], T, P), np.int64)
                # per-half-slot source ids for the post-gather norm multiply
                for tt in range(T):
                    for p in range(P):
                        kp = rd["npair"][tt, p]
                        first = srow(rd["ptok"][0, tt, p]) if kp > 0 else 0
                        for s in range(rP2[ri]):
                            pl[s, tt, p] = (srow(rd["ptok"][s, tt, p])
                                            if s < kp else first)
                        t = rlo + tt
                        if vmc[p, t] and deg[ids[p, t]] >= 1:
                            pcp[p, t] = rP2[ri] - kp
                            pcm1[p, t] += rd["seeded"][tt, p]
                pidx_parts.append(_wrap16(pl.reshape(-1)))
                ph = np.stack([srcof[pl], srcof[pl + 1]], axis=-1)
                nxid_parts.append(ph.reshape(rP2[ri] * T, P * 2)
                                  .reshape(rP2[ri] * T, P, 2)
                                  .transpose(1, 0, 2).reshape(P, -1))
            if rS1[ri] > 0:
                sl = np.zeros((rS1[ri], T, P), np.int64)
                for tt in range(T):
                    for p in range(P):
                        ks = rd["nsing"][tt, p]
                        kp = rd["npair"][tt, p]
                        if ks > 0:
                            fb = srow(rd["stok"][0, tt, p])
                        elif kp > 0:
                            fb = srow(rd["ptok"][0, tt, p])
                        else:
                            fb = 0
                        for s in range(rS1[ri]):
                            sl[s, tt, p] = (srow(rd["stok"][s, tt, p])
                                            if s < ks else fb)
                        t = rlo + tt
                        if vmc[p, t] and deg[ids[p, t]] >= 1:
                            if ks > 0:
                                pcs1[p, t] = rS1[ri] - ks
                            else:
                                pcm1[p, t] += rS1[ri]
                sidx_parts.append(_wrap16(sl.reshape(-1)))
                nxid_parts.append(srcof[sl].transpose(2, 0, 1)
                                  .reshape(P, -1))

        d = np.where(vmc, deg[ids], 0).astype(np.float32)
        meta = np.concatenate(
            [d, vmc.astype(np.float32), pcp, pcm1, pcs1], axis=1)
        per_core.append(dict(
            cidx=np.concatenate(cidx_parts, axis=1),
            pidx=(np.concatenate(pidx_parts, axis=1) if pidx_parts
                  else np.zeros((P, L16), np.int16)),
            sidx=(np.concatenate(sidx_parts, axis=1) if sidx_parts
                  else np.zeros((P, L16), np.int16)),
            nxid=np.concatenate(nxid_parts, axis=1),
            meta=meta.astype(np.float32), ids=ids, vmask=vmc,
            l1cnt=l1cnt))

    return dict(nt=nt, rounds=rounds, rP2=rP2, rS1=rS1,
                ncols_rq=ncols_rq, per_core=per_core)


# -------------------------------------------------------------- device side
def _build_program(sched):
    import concourse.bass as bass
    import concourse.tile as tile
    from concourse import bacc, mybir

    f32 = mybir.dt.float32
    i16 = mybir.dt.int16
    Alu = mybir.AluOpType
    Act = mybir.ActivationFunctionType
    AxX = mybir.AxisListType.X

    nt = sched["nt"]
    rounds = sched["rounds"]
    rP2 = sched["rP2"]
    rS1 = sched["rS1"]
    ncols_rq = sched["ncols_rq"]
    nrounds = len(rounds)
    pc0 = sched["per_core"][0]
    ccols = pc0["cidx"].shape[1]
    pcols = pc0["pidx"].shape[1]
    scols = pc0["sidx"].shape[1]
    nxcols = pc0["nxid"].shape[1]
    Tmax = max(hi - lo for lo, hi in rounds)
    ncmax = int(ncols_rq.max())
    PTmax = max(rP2[i] * (rounds[i][1] - rounds[i][0])
                for i in range(nrounds))
    STmax = max(max(rS1[i], 1) * (rounds[i][1] - rounds[i][0])
                for i in range(nrounds))

    nc = bacc.Bacc("TRN2", target_bir_lowering=False, debug=False,
                   num_devices=NCORES, num_swdge_queues=NQUEUES)

    h_d = nc.dram_tensor("h_in", [N_NODES, FEAT], f32,
                         kind="ExternalInput").ap()
    hown_d = nc.dram_tensor("hown_in", [P, nt * FEAT], f32,
                            kind="ExternalInput").ap()
    nown_d = nc.dram_tensor("nown_in", [P, nt], f32,
                            kind="ExternalInput").ap()
    nx_d = nc.dram_tensor("nx_in", [P, nxcols], f32,
                          kind="ExternalInput").ap()
    bnw_d = nc.dram_tensor("bnw_in", [FEAT], f32, kind="ExternalInput").ap()
    bnb_d = nc.dram_tensor("bnb_in", [FEAT], f32, kind="ExternalInput").ap()
    cidx_d = nc.dram_tensor("cidx_in", [P, ccols], i16,
                            kind="ExternalInput").ap()
    pidx_d = nc.dram_tensor("pidx_in", [P, pcols], i16,
                            kind="ExternalInput").ap()
    sidx_d = nc.dram_tensor("sidx_in", [P, scols], i16,
                            kind="ExternalInput").ap()
    meta_d = nc.dram_tensor("meta_in", [P, 5 * nt], f32,
                            kind="ExternalInput").ap()
    cnt_d = nc.dram_tensor("cnt_in", [P, nrounds * (NQ + 2)],
                       mybir.dt.int32,
                           kind="ExternalInput").ap()
    out_d = nc.dram_tensor("out", [P, nt * FEAT], f32,
                           kind="ExternalOutput").ap()

    qcount = [0]

    def nextq():
        q = qcount[0] % NQUEUES
        qcount[0] += 1
        return q

    with tile.TileContext(nc) as tc:
        with (
            tc.tile_pool(name="dram", bufs=1, space="DRAM") as dpool,
            tc.tile_pool(name="stagp", bufs=2, space="DRAM") as stpoold,
            tc.tile_pool(name="shdram", bufs=1, space="DRAM") as shpool,
            tc.tile_pool(name="const", bufs=1) as cpool,
            tc.tile_pool(name="scal", bufs=1) as spool,
            tc.tile_pool(name="cg", bufs=2) as cgpool,
            tc.tile_pool(name="gp", bufs=2) as gppool,
            tc.tile_pool(name="gs", bufs=2) as gspool,
            tc.tile_pool(name="idxp", bufs=2) as idxpool,
            tc.tile_pool(name="nxp", bufs=2) as nxpool,
            tc.tile_pool(name="stats", bufs=1) as stpool,
            tc.tile_pool(name="bm", bufs=6) as bmpool,
            tc.tile_pool(name="outr", bufs=2) as orpool,
            tc.tile_pool(name="bnp", bufs=1) as bnpool,
            tc.tile_pool(name="psum", bufs=2, space="PSUM") as pspool,
        ):
            out_pre = dpool.tile([P, nt * FEAT], f32, name="out_pre")
            cc_in = dpool.tile([1, 2 * FEAT], f32)
            cc_out = shpool.tile([1, 2 * FEAT], f32, addr_space="Shared")

            # ---- metas + per-node scalars ----
            meta_sb = cpool.tile([P, 5 * nt], f32)
            nc.sync.dma_start(out=meta_sb, in_=meta_d)
            cnt_sb = cpool.tile([P, nrounds * (NQ + 2)], mybir.dt.int32)
            nc.sync.dma_start(out=cnt_sb, in_=cnt_d)
            degf = meta_sb[:, 0:nt]
            valid = meta_sb[:, nt:2 * nt]
            pcp = meta_sb[:, 2 * nt:3 * nt]
            pcm1 = meta_sb[:, 3 * nt:4 * nt]
            pcs1 = meta_sb[:, 4 * nt:5 * nt]
            nown = cpool.tile([P, nt], f32)
            nc.sync.dma_start(out=nown, in_=nown_d)

            eps_std = spool.tile([P, 1], f32)
            nc.vector.memset(eps_std, EPS_STD)
            eps_bn = spool.tile([P, 1], f32)
            nc.vector.memset(eps_bn, EPS_BN)

            ds = spool.tile([P, nt], f32)
            nc.vector.tensor_scalar_max(out=ds, in0=degf, scalar1=1.0)
            rdeg = spool.tile([P, nt], f32)
            nc.vector.reciprocal(out=rdeg, in_=ds)
            hb = spool.tile([P, nt], f32)
            nc.vector.tensor_scalar(out=hb, in0=degf, scalar1=0.0,
                                    scalar2=None, op0=Alu.is_gt)
            logd = spool.tile([P, nt], f32)
            nc.scalar.activation(out=logd, in_=degf, func=Act.Ln,
                                 bias=1.0, scale=1.0)
            lsafe = spool.tile([P, nt], f32)
            nc.vector.tensor_scalar_max(out=lsafe, in0=logd, scalar1=0.5)
            rlog = spool.tile([P, nt], f32)
            nc.vector.reciprocal(out=rlog, in_=lsafe)
            t1 = spool.tile([P, nt], f32)
            nc.vector.tensor_scalar(out=t1, in0=logd, scalar1=1.0 / AVG_D_LOG,
                                    scalar2=1.0, op0=Alu.mult, op1=Alu.add)
            t2 = spool.tile([P, nt], f32)
            nc.vector.tensor_scalar_mul(out=t2, in0=rlog, scalar1=AVG_D_LOG)
            sS = spool.tile([P, nt], f32)
            nc.vector.tensor_tensor(out=sS, in0=t1, in1=t2, op=Alu.add)
            t3 = spool.tile([P, nt], f32)
            nc.vector.tensor_tensor(out=t3, in0=nown, in1=sS, op=Alu.mult)
            t4 = spool.tile([P, nt], f32)
            nc.vector.tensor_tensor(out=t4, in0=t3, in1=hb, op=Alu.mult)
            tpre = spool.tile([P, nt], f32)
            nc.vector.tensor_scalar_mul(out=tpre, in0=t4, scalar1=1.0 / 13.0)
            c1 = spool.tile([P, nt], f32)
            nc.vector.tensor_scalar_mul(out=c1, in0=valid, scalar1=1.0 / 13.0)

            rs1 = bnpool.tile([P, FEAT], f32)
            rs2 = bnpool.tile([P, FEAT], f32)
            nc.vector.memset(rs1, 0.0)
            nc.vector.memset(rs2, 0.0)

            # per-round offsets into concatenated index/nx arrays
            cro, pro, sro, nxo = [], [], [], []
            cpos = ppos = spos = nxpos = 0
            for ri in range(nrounds):
                T = rounds[ri][1] - rounds[ri][0]
                clen = int(sum(ncols_rq[ri, q] * P // L16 for q in range(NQ)))
                cro.append(cpos)
                cpos += clen
                plen = (rP2[ri] * T * P // L16) if rP2[ri] > 0 else 0
                pro.append(ppos)
                ppos += plen
                slen = (rS1[ri] * T * P // L16) if rS1[ri] > 0 else 0
                sro.append(spos)
                spos += slen
                nxo.append(nxpos)
                nxpos += 2 * rP2[ri] * T + rS1[ri] * T

            CGW = ncmax * FEAT
            creg = nc.gpsimd.alloc_register("l1cnt_reg")
            stagmax = int(max(ncols_rq[ri].sum() for ri in range(nrounds))
                          ) * P

            def emit_l1(ri):
                cidx_sb = idxpool.tile([P, ncmax * NQ * P // L16], i16,
                                       tag="cidx", name=f"cidx{ri}")
                clen = int(sum(ncols_rq[ri, q] * P // L16 for q in range(NQ)))
                nc.sync.dma_start(out=cidx_sb[:, :clen],
                                  in_=cidx_d[:, cro[ri]:cro[ri] + clen])
                cgs = []
                cbase = 0
                for q in range(NQ):
                    ncols = int(ncols_rq[ri, q])
                    ni = ncols * P
                    CG = cgpool.tile([P, CGW], f32, tag=f"CG{q}",
                                     name=f"CG{ri}_{q}")
                    k = ri * (NQ + 2) + q
                    nc.gpsimd.reg_load(creg, cnt_sb[0:1, k:k + 1])
                    nc.gpsimd.dma_gather(
                        out_ap=CG[:, :ncols * FEAT].rearrange(
                            "p (c f) -> p c f", f=FEAT),
                        in_ap=bass.AP(tensor=h_d.tensor,
                                      offset=q * QROWS * FEAT,
                                      ap=[[FEAT, QROWS], [1, FEAT]]),
                        idxs_ap=cidx_sb[:, cbase:cbase + ni // L16],
                        num_idxs=ni, num_idxs_reg=creg, elem_size=FEAT,
                        single_packet=False, queue_num=nextq())
                    cbase += ni // L16
                    cgs.append(CG)
                return cgs

            def emit_compute(ri, cgs):
                rlo, rhi = rounds[ri]
                T = rhi - rlo
                S2 = rP2[ri]
                S1 = rS1[ri]
                cb = [0]
                for q in range(NQ):
                    cb.append(cb[-1] + int(ncols_rq[ri, q]) * P)
                rows_used = cb[-1]

                # j-major restage (raw h rows; norm applied post-L2)
                nxw = 2 * S2 * T + S1 * T
                nxsb = nxpool.tile([P, (2 * PTmax + STmax)], f32, tag="nx",
                                   name=f"nx{ri}")
                nc.sync.dma_start(out=nxsb[:, :nxw],
                                  in_=nx_d[:, nxo[ri]:nxo[ri] + nxw])
                stag = stpoold.tile([stagmax, FEAT], f32, tag="stag",
                                    name=f"stag{ri}")
                for q in range(NQ):
                    ncols = int(ncols_rq[ri, q])
                    nc.sync.dma_start(
                        out=bass.AP(tensor=stag.tensor,
                                    offset=stag.offset + cb[q] * FEAT,
                                    ap=[[FEAT, P], [P * FEAT, ncols],
                                        [1, FEAT]]),
                        in_=cgs[q][:, :ncols * FEAT])

                # L2 gathers
                Gp = Gs = None
                if S2 > 0:
                    ni = S2 * T * P
                    pidx_sb = idxpool.tile([P, PTmax * P // L16], i16,
                                           tag="pidx", name=f"pidx{ri}")
                    nc.sync.dma_start(
                        out=pidx_sb[:, :ni // L16],
                        in_=pidx_d[:, pro[ri]:pro[ri] + ni // L16])
                    Gp = gppool.tile([P, PTmax * 2 * FEAT], f32, tag="Gp",
                                     name=f"Gp{ri}")
                    kk = ri * (NQ + 2) + NQ
                    nc.gpsimd.reg_load(creg, cnt_sb[0:1, kk:kk + 1])
                    nc.gpsimd.dma_gather(
                        out_ap=Gp[:, :S2 * T * 2 * FEAT].rearrange(
                            "p (c e) -> p c e", e=2 * FEAT),
                        in_ap=bass.AP(tensor=stag.tensor, offset=stag.offset,
                                      ap=[[FEAT, rows_used - 1],
                                          [1, 2 * FEAT]]),
                        idxs_ap=pidx_sb[:, :ni // L16],
                        num_idxs=ni, num_idxs_reg=creg, elem_size=2 * FEAT,
                        elem_step=FEAT, single_packet=False,
                        queue_num=nextq())
                if S1 > 0:
                    ni = S1 * T * P
                    sidx_sb = idxpool.tile([P, STmax * P // L16], i16,
                                           tag="sidx", name=f"sidx{ri}")
                    nc.sync.dma_start(
                        out=sidx_sb[:, :ni // L16],
                        in_=sidx_d[:, sro[ri]:sro[ri] + ni // L16])
                    Gs = gspool.tile([P, STmax * FEAT], f32, tag="Gs",
                                     name=f"Gs{ri}")
                    kk = ri * (NQ + 2) + NQ + 1
                    nc.gpsimd.reg_load(creg, cnt_sb[0:1, kk:kk + 1])
                    nc.gpsimd.dma_gather(
                        out_ap=Gs[:, :S1 * T * FEAT].rearrange(
                            "p (c f) -> p c f", f=FEAT),
                        in_ap=bass.AP(tensor=stag.tensor, offset=stag.offset,
                                      ap=[[FEAT, rows_used], [1, FEAT]]),
                        idxs_ap=sidx_sb[:, :ni // L16],
                        num_idxs=ni, num_idxs_reg=creg, elem_size=FEAT,
                        single_packet=False, queue_num=nextq())

                # norm[src] multiply on gathered data (in place)
                nc.vector.tensor_tensor(
                    out=Gp[:, :S2 * T * 2 * FEAT].rearrange(
                        "p (c f) -> p c f", f=FEAT),
                    in0=Gp[:, :S2 * T * 2 * FEAT].rearrange(
                        "p (c f) -> p c f", f=FEAT),
                    in1=nxsb[:, :2 * S2 * T].to_broadcast(
                        [P, 2 * S2 * T, FEAT]),
                    op=Alu.mult)
                if S1 > 0:
                    nc.vector.tensor_tensor(
                        out=Gs[:, :S1 * T * FEAT].rearrange(
                            "p (c f) -> p c f", f=FEAT),
                        in0=Gs[:, :S1 * T * FEAT].rearrange(
                            "p (c f) -> p c f", f=FEAT),
                        in1=nxsb[:, 2 * S2 * T:2 * S2 * T + S1 * T]
                        .to_broadcast([P, S1 * T, FEAT]),
                        op=Alu.mult)

                # ---- reduces ----
                TF = T * FEAT
                s1p = stpool.tile([P, Tmax * 2 * FEAT], f32, tag="s1p",
                                  name=f"s1p{ri}")
                s2p = stpool.tile([P, Tmax * 2 * FEAT], f32, tag="s2p",
                                  name=f"s2p{ri}")
                mxp = stpool.tile([P, Tmax * 2 * FEAT], f32, tag="mxp",
                                  name=f"mxp{ri}")
                mnp = stpool.tile([P, Tmax * 2 * FEAT], f32, tag="mnp",
                                  name=f"mnp{ri}")
                s1s = stpool.tile([P, Tmax * FEAT], f32, tag="s1s",
                                  name=f"s1s{ri}")
                s2s = stpool.tile([P, Tmax * FEAT], f32, tag="s2s",
                                  name=f"s2s{ri}")
                mxs = stpool.tile([P, Tmax * FEAT], f32, tag="mxs",
                                  name=f"mxs{ri}")
                mns = stpool.tile([P, Tmax * FEAT], f32, tag="mns",
                                  name=f"mns{ri}")

                assert S2 > 0
                gp3 = Gp[:, :S2 * T * 2 * FEAT].rearrange(
                    "p (s c) -> p c s", s=S2)
                nc.vector.tensor_reduce(out=s1p[:, :T * 2 * FEAT], in_=gp3,
                                        axis=AxX, op=Alu.add)
                nc.vector.tensor_reduce(out=mxp[:, :T * 2 * FEAT], in_=gp3,
                                        axis=AxX, op=Alu.max)
                nc.vector.tensor_reduce(out=mnp[:, :T * 2 * FEAT], in_=gp3,
                                        axis=AxX, op=Alu.min)
                if S1 > 0:
                    gs3 = Gs[:, :S1 * TF].rearrange(
                        "p (s c) -> p c s", s=S1)
                    nc.vector.tensor_reduce(out=s1s[:, :TF], in_=gs3,
                                            axis=AxX, op=Alu.add)
                    nc.vector.tensor_reduce(out=mxs[:, :TF], in_=gs3,
                                            axis=AxX, op=Alu.max)
                    nc.vector.tensor_reduce(out=mns[:, :TF], in_=gs3,
                                            axis=AxX, op=Alu.min)

                # first-pair / first-single views (pre-square values)
                g0 = Gp[:, :T * 2 * FEAT].rearrange("p (t h) -> p t h",
                                                    h=2 * FEAT)
                m1 = g0[:, :, 0:FEAT]
                m2 = g0[:, :, FEAT:2 * FEAT]
                sg0 = (Gs[:, :TF].rearrange("p (t f) -> p t f", f=FEAT)
                       if S1 > 0 else None)

                _bmn = [0]

                def bm():
                    _bmn[0] += 1
                    return bmpool.tile([P, Tmax * FEAT], f32, tag="bm",
                                       name=f"bm_{ri}_{_bmn[0]}")

                bc = lambda ap: ap[:, rlo:rhi].to_broadcast([P, T, FEAT])
                r3 = lambda ap: ap[:, :TF].rearrange("p (t f) -> p t f",
                                                     f=FEAT)
                sp3 = lambda ap, off: ap[:, :T * 2 * FEAT].rearrange(
                    "p (t h) -> p t h", h=2 * FEAT)[:, :, off:off + FEAT]

                # s1 combine + corrections (uses pre-square Gp/Gs views)
                s1 = stpool.tile([P, Tmax * FEAT], f32, tag="s1c",
                                 name=f"s1c{ri}")
                nc.vector.tensor_tensor(out=r3(s1), in0=sp3(s1p, 0),
                                        in1=sp3(s1p, FEAT), op=Alu.add)
                if S1 > 0:
                    nc.vector.tensor_tensor(out=s1[:, :TF], in0=s1[:, :TF],
                                            in1=s1s[:, :TF], op=Alu.add)
                u12 = bm()
                nc.vector.tensor_tensor(out=r3(u12), in0=m1, in1=m2,
                                        op=Alu.add)
                cA = bm()
                nc.vector.tensor_tensor(out=r3(cA), in0=r3(u12),
                                        in1=bc(pcp), op=Alu.mult)
                nc.vector.tensor_tensor(out=s1[:, :TF], in0=s1[:, :TF],
                                        in1=cA[:, :TF], op=Alu.subtract)
                cB = bm()
                nc.vector.tensor_tensor(out=r3(cB), in0=m1, in1=bc(pcm1),
                                        op=Alu.mult)
                nc.vector.tensor_tensor(out=s1[:, :TF], in0=s1[:, :TF],
                                        in1=cB[:, :TF], op=Alu.subtract)
                if S1 > 0:
                    cC = bm()
                    nc.vector.tensor_tensor(out=r3(cC), in0=sg0,
                                            in1=bc(pcs1), op=Alu.mult)
                    nc.vector.tensor_tensor(out=s1[:, :TF], in0=s1[:, :TF],
                                            in1=cC[:, :TF], op=Alu.subtract)

                # max / min combine
                mx = stpool.tile([P, Tmax * FEAT], f32, tag="mxc",
                                 name=f"mxc{ri}")
                nc.vector.tensor_tensor(out=r3(mx), in0=sp3(mxp, 0),
                                        in1=sp3(mxp, FEAT), op=Alu.max)
                if S1 > 0:
                    nc.vector.tensor_tensor(out=mx[:, :TF], in0=mx[:, :TF],
                                            in1=mxs[:, :TF], op=Alu.max)
                mn = stpool.tile([P, Tmax * FEAT], f32, tag="mnc",
                                 name=f"mnc{ri}")
                nc.vector.tensor_tensor(out=r3(mn), in0=sp3(mnp, 0),
                                        in1=sp3(mnp, FEAT), op=Alu.min)
                if S1 > 0:
                    nc.vector.tensor_tensor(out=mn[:, :TF], in0=mn[:, :TF],
                                            in1=mns[:, :TF], op=Alu.min)

                # square in place, then sumsq + corrections
                nc.scalar.activation(out=Gp[:, :S2 * T * 2 * FEAT],
                                     in_=Gp[:, :S2 * T * 2 * FEAT],
                                     func=Act.Square)
                nc.vector.tensor_reduce(out=s2p[:, :T * 2 * FEAT], in_=gp3,
                                        axis=AxX, op=Alu.add)
                if S1 > 0:
                    nc.scalar.activation(out=Gs[:, :S1 * TF],
                                         in_=Gs[:, :S1 * TF],
                                         func=Act.Square)
                    nc.vector.tensor_reduce(out=s2s[:, :TF], in_=gs3,
                                            axis=AxX, op=Alu.add)
                s2 = stpool.tile([P, Tmax * FEAT], f32, tag="s2c",
                                 name=f"s2c{ri}")
                nc.vector.tensor_tensor(out=r3(s2), in0=sp3(s2p, 0),
                                        in1=sp3(s2p, FEAT), op=Alu.add)
                if S1 > 0:
                    nc.vector.tensor_tensor(out=s2[:, :TF], in0=s2[:, :TF],
                                            in1=s2s[:, :TF], op=Alu.add)
                uq = bm()
                nc.vector.tensor_tensor(out=r3(uq), in0=m1, in1=m2,
                                        op=Alu.add)
                cD = bm()
                nc.vector.tensor_tensor(out=r3(cD), in0=r3(uq),
                                        in1=bc(pcp), op=Alu.mult)
                nc.vector.tensor_tensor(out=s2[:, :TF], in0=s2[:, :TF],
                                        in1=cD[:, :TF], op=Alu.subtract)
                cE = bm()
                nc.vector.tensor_tensor(out=r3(cE), in0=m1, in1=bc(pcm1),
                                        op=Alu.mult)
                nc.vector.tensor_tensor(out=s2[:, :TF], in0=s2[:, :TF],
                                        in1=cE[:, :TF], op=Alu.subtract)
                if S1 > 0:
                    cF = bm()
                    nc.vector.tensor_tensor(out=r3(cF), in0=sg0,
                                            in1=bc(pcs1), op=Alu.mult)
                    nc.vector.tensor_tensor(out=s2[:, :TF], in0=s2[:, :TF],
                                            in1=cF[:, :TF], op=Alu.subtract)

                # ---- per-node math ----
                mean = bm()
                nc.vector.tensor_tensor(out=r3(mean), in0=r3(s1),
                                        in1=bc(rdeg), op=Alu.mult)
                e2 = bm()
                nc.vector.tensor_tensor(out=r3(e2), in0=r3(s2),
                                        in1=bc(rdeg), op=Alu.mult)
                msq = bm()
                nc.scalar.activation(out=msq[:, :TF], in_=mean[:, :TF],
                                     func=Act.Square)
                varr = bm()
                nc.vector.tensor_tensor(out=varr[:, :TF], in0=e2[:, :TF],
                                        in1=msq[:, :TF], op=Alu.subtract)
                var0 = bm()
                nc.vector.tensor_scalar_max(out=var0[:, :TF],
                                            in0=varr[:, :TF], scalar1=0.0)
                stdv = bm()
                nc.scalar.activation(out=stdv[:, :TF], in_=var0[:, :TF],
                                     func=Act.Sqrt, bias=eps_std, scale=1.0)
                pre1 = bm()
                nc.vector.tensor_tensor(out=pre1[:, :TF], in0=mean[:, :TF],
                                        in1=mx[:, :TF], op=Alu.add)
                pre2 = bm()
                nc.vector.tensor_tensor(out=pre2[:, :TF], in0=mn[:, :TF],
                                        in1=stdv[:, :TF], op=Alu.add)
                pre = bm()
                nc.vector.tensor_tensor(out=pre[:, :TF], in0=pre1[:, :TF],
                                        in1=pre2[:, :TF], op=Alu.add)
                v2 = bm()
                nc.vector.tensor_tensor(out=r3(v2), in0=r3(pre),
                                        in1=bc(tpre), op=Alu.mult)
                hok = bm()
                nc.sync.dma_start(out=hok[:, :TF],
                                  in_=hown_d[:, rlo * FEAT:rhi * FEAT])
                hno = bm()
                nc.vector.tensor_tensor(out=r3(hno), in0=r3(hok),
                                        in1=bc(nown), op=Alu.mult)
                v1 = bm()
                nc.vector.tensor_tensor(out=r3(v1), in0=r3(hno),
                                        in1=bc(c1), op=Alu.mult)
                outp = bm()
                nc.vector.tensor_tensor(out=outp[:, :TF], in0=v1[:, :TF],
                                        in1=v2[:, :TF], op=Alu.add)
                out_r = orpool.tile([P, Tmax * FEAT], f32, tag="outr",
                                    name=f"outr{ri}")
                nc.scalar.activation(out=out_r[:, :TF],
                                     in_=outp[:, :TF], func=Act.Relu)
                nc.sync.dma_start(out=out_pre[:, rlo * FEAT:rhi * FEAT],
                                  in_=out_r[:, :TF])

                orv = out_r[:, :TF].rearrange("p (t f) -> p f t", f=FEAT)
                rst = bm()
                nc.vector.tensor_reduce(out=rst[:, :FEAT], in_=orv,
                                        axis=AxX, op=Alu.add)
                nc.vector.tensor_tensor(out=rs1, in0=rs1,
                                        in1=rst[:, :FEAT], op=Alu.add)
                sqr = bm()
                nc.scalar.activation(out=sqr[:, :TF], in_=out_r[:, :TF],
                                     func=Act.Square)
                rst2 = bm()
                nc.vector.tensor_reduce(
                    out=rst2[:, :FEAT],
                    in_=sqr[:, :TF].rearrange("p (t f) -> p f t", f=FEAT),
                    axis=AxX, op=Alu.add)
                nc.vector.tensor_tensor(out=rs2, in0=rs2,
                                        in1=rst2[:, :FEAT], op=Alu.add)

            # ---- main pipeline: L1(r+1) before compute(r) ----
            cgs_cur = emit_l1(0)
            for ri in range(nrounds):
                cgs_next = emit_l1(ri + 1) if ri + 1 < nrounds else None
                emit_compute(ri, cgs_cur)
                cgs_cur = cgs_next

            # ---- BatchNorm ----
            ones = bnpool.tile([P, 1], f32)
            nc.vector.memset(ones, 1.0)
            rsboth = bnpool.tile([P, 2 * FEAT], f32)
            nc.vector.tensor_copy(out=rsboth[:, :FEAT], in_=rs1)
            nc.vector.tensor_copy(out=rsboth[:, FEAT:], in_=rs2)
            ps = pspool.tile([P, 2 * FEAT], f32, tag="ps")
            nc.tensor.matmul(out=ps[:1, :], lhsT=ones, rhs=rsboth,
                             start=True, stop=True)
            stats_sb = bnpool.tile([P, 2 * FEAT], f32)
            nc.vector.tensor_copy(out=stats_sb[:1, :], in_=ps[:1, :])
            nc.sync.dma_start(out=cc_in, in_=stats_sb[:1, :])
            nc.gpsimd.collective_compute(
                "AllReduce", mybir.AluOpType.add,
                replica_groups=[list(range(NCORES))],
                ins=[cc_in.opt()], outs=[cc_out.opt()])
            gl = bnpool.tile([P, 2 * FEAT], f32)
            nc.sync.dma_start(out=gl[:1, :], in_=cc_out)

            mu = bnpool.tile([P, FEAT], f32)
            nc.vector.tensor_scalar_mul(out=mu[:1, :], in0=gl[:1, :FEAT],
                                        scalar1=1.0 / N_NODES)
            em2 = bnpool.tile([P, FEAT], f32)
            nc.vector.tensor_scalar_mul(out=em2[:1, :], in0=gl[:1, FEAT:],
                                        scalar1=1.0 / N_NODES)
            musq = bnpool.tile([P, FEAT], f32)
            nc.scalar.activation(out=musq[:1, :], in_=mu[:1, :],
                                 func=Act.Square)
            varb = bnpool.tile([P, FEAT], f32)
            nc.vector.tensor_tensor(out=varb[:1, :], in0=em2[:1, :],
                                    in1=musq[:1, :], op=Alu.subtract)
            stdb = bnpool.tile([P, FEAT], f32)
            nc.scalar.activation(out=stdb[:1, :], in_=varb[:1, :],
                                 func=Act.Sqrt, bias=eps_bn[:1], scale=1.0)
            rstd = bnpool.tile([P, FEAT], f32)
            nc.vector.reciprocal(out=rstd[:1, :], in_=stdb[:1, :])
            bnw_sb = bnpool.tile([P, FEAT], f32)
            nc.sync.dma_start(out=bnw_sb[:1, :], in_=bnw_d[None, :])
            bnb_sb = bnpool.tile([P, FEAT], f32)
            nc.sync.dma_start(out=bnb_sb[:1, :], in_=bnb_d[None, :])
            scsh = bnpool.tile([P, 2 * FEAT], f32)
            nc.vector.tensor_tensor(out=scsh[:1, :FEAT], in0=rstd[:1, :],
                                    in1=bnw_sb[:1, :], op=Alu.mult)
            msc = bnpool.tile([P, FEAT], f32)
            nc.vector.tensor_tensor(out=msc[:1, :], in0=mu[:1, :],
                                    in1=scsh[:1, :FEAT], op=Alu.mult)
            nc.vector.tensor_tensor(out=scsh[:1, FEAT:], in0=bnb_sb[:1, :],
                                    in1=msc[:1, :], op=Alu.subtract)
            onesr = bnpool.tile([P, P], f32)
            nc.vector.memset(onesr[:1, :], 1.0)
            psb = pspool.tile([P, 2 * FEAT], f32, tag="psb")
            nc.tensor.matmul(out=psb, lhsT=onesr[:1, :], rhs=scsh[:1, :],
                             start=True, stop=True)
            scsh_b = bnpool.tile([P, 2 * FEAT], f32)
            nc.vector.tensor_copy(out=scsh_b, in_=psb)

            def _mid_bcast(ap2d, n):
                return bass.AP(tensor=ap2d.tensor, offset=ap2d.offset,
                               ap=[ap2d.ap[0], [0, n], ap2d.ap[1]])

            CH = 14
            for clo in range(0, nt, CH):
                chi = min(clo + CH, nt)
                nf2 = (chi - clo) * FEAT
                ob = orpool.tile([P, CH * FEAT], f32, tag="obn",
                                 name=f"obn{clo}")
                nc.sync.dma_start(out=ob[:, :nf2],
                                  in_=out_pre[:, clo * FEAT:chi * FEAT])
                o3 = ob[:, :nf2].rearrange("p (t f) -> p t f", f=FEAT)
                nc.vector.tensor_tensor(
                    out=o3, in0=o3,
                    in1=_mid_bcast(scsh_b[:, :FEAT], chi - clo),
                    op=Alu.mult)
                nc.vector.tensor_tensor(
                    out=o3, in0=o3,
                    in1=_mid_bcast(scsh_b[:, FEAT:], chi - clo),
                    op=Alu.add)
                nc.sync.dma_start(out=out_d[:, clo * FEAT:chi * FEAT],
                                  in_=ob[:, :nf2])

    nc.compile()
    return nc


# ---------------------------------------------------------------- entrypoint
def kernel(h, norm, e, bn_weight, bn_bias, src, dst, **_ignored):
    global LAST_RESULTS
    from concourse import bass_utils

    h = np.ascontiguousarray(np.asarray(h, dtype=np.float32))
    norm = np.ascontiguousarray(np.asarray(norm, dtype=np.float32))
    bn_weight = np.ascontiguousarray(np.asarray(bn_weight, dtype=np.float32))
    bn_bias = np.ascontiguousarray(np.asarray(bn_bias, dtype=np.float32))
    src_i = np.asarray(src)
    dst_i = np.asarray(dst)
    assert h.shape == (N_NODES, FEAT) and src_i.shape == (N_EDGES,)

    key = (int(np.bitwise_xor.reduce(src_i.view(np.uint32))),
           int(np.bitwise_xor.reduce(dst_i.view(np.uint32))))
    if key in _CACHE:
        sched, nc = _CACHE[key]
    else:
        sched = _build_schedule(src_i, dst_i)
        nc = _build_program(sched)
        _CACHE[key] = (sched, nc)

    nt = sched["nt"]
    norm1 = norm[:, 0]
    in_maps = []
    for c in range(NCORES):
        pc = sched["per_core"][c]
        ids = pc["ids"]
        in_maps.append({
            "h_in": h,
            "hown_in": np.ascontiguousarray(
                h[ids].reshape(P, nt * FEAT)),
            "nown_in": np.ascontiguousarray(norm1[ids]).astype(np.float32),
            "nx_in": np.ascontiguousarray(norm1[pc["nxid"]]).astype(
                np.float32),
            "bnw_in": bn_weight,
            "bnb_in": bn_bias,
            "cidx_in": pc["cidx"],
            "pidx_in": pc["pidx"],
            "sidx_in": pc["sidx"],
            "meta_in": pc["meta"],
            "cnt_in": np.ascontiguousarray(
                np.tile(pc["l1cnt"].reshape(1, -1), (P, 1))),
        })

    trace = bool(int(os.environ.get("KERNEL_TRACE", "0")))
    res = bass_utils.run_bass_kernel_spmd(
        nc, in_maps, core_ids=list(range(NCORES)), trace=trace)
    LAST_RESULTS = res

    out_full = np.empty((N_NODES, FEAT), np.float32)
    for c in range(NCORES):
        pc = sched["per_core"][c]
        arr = np.asarray(res.results[c]["out"]).reshape(P, nt, FEAT)
        vm = pc["vmask"]
        out_full[pc["ids"][vm]] = arr[vm]
    return out_full
